# revision 1
# baseline (speedup 1.0000x reference)
"""Trainium2 Bass kernel for nn_Model_11888469475921 (dense_cnn).

Computation (per image, 1024 images total):
  proj = conv3x3(x, weight) + bias          # (64, 32, 32), padding 1
  act  = selu(proj)
  pooled = sqrt(act[...,0::2]^2 + act[...,1::2]^2)   # LPPool1d(p=2,k=2) along W
  gate = sigmoid(mean_{H,W}(x) @ scale_proj + scale_bias)
  out  = mean_{H,Wp}(pooled) * gate         # (64,)

Strategy: pure data parallel over 8 NeuronCores (128 images each).

Conv: ki-only unfold + kj as matmul free-offsets. A per-chunk patch tensor
[19, 32*CH_PAIRS*34] bf16 holds x[c, i+ki-1, row] for partition k=9r+3ki+c
(r = image half of the pair, row kept 34-wide so the kj shift is a free-dim
offset). Three PSUM-accumulating matmuls per 512-pixel parity group with
block-diagonal lhsT [19, 128] (two images per matmul; ones-row 18 carries the
conv bias on the kj=1 plane). Weights are host-packed, pre-scaled by 1/alpha.

SELU^2 without a selu table, one exp pass, true square (no cancellation):
  PSUM holds a~ = a/alpha
  e = Exp(alpha * a~)                  (ACT, PSUM->SBUF bf16)
  m = min(e - 1, 0)                    (DVE tensor_scalar, 4x bf16)
  r = Relu(a~)   (ACT for 3/4 of pairs; DVE scalar_tensor_tensor otherwise,
                  balancing ACT vs DVE occupancy)
  t = r + m  = selu(a)/(s*alpha)       (DVE)
  u = t * t                            (DVE tensor_tensor)
  q = u_even + u_odd                   (DVE; parity halves are the two
                                        512-col halves of the PSUM pair tile)
  res[:, g] = accum_out of ACT Sqrt((s*alpha/512)^2 * q)  (fused pool+mean)

Gate: channel sums via row-reduce + selector matmul, gate matmuls on PE,
sigmoid as 1/(1+exp(-x)) with DVE reciprocal (stays in the exp table set).

Phases: setup / main / sqrt live in separate TileContexts: the exit barriers
keep every DMA at <=1 sync wait and keep all Exp ACT ops before all Sqrt ops
(one activation-table switch total). _split_multiwait() post-processes the
BIR because this walrus build can only codegen one sync-wait per instruction.

Output rows are stored in (r, chan) x (chunk, p) order and permuted on host.
"""

import os
import numpy as np
import ml_dtypes
from contextlib import ExitStack

import bass_rust
import concourse.bass as bass
import concourse.mybir as mybir
from concourse.tile import TileContext
from concourse.bass_utils import run_bass_kernel_spmd

AF = mybir.ActivationFunctionType
ALU = mybir.AluOpType
AX = mybir.AxisListType
F32 = mybir.dt.float32
BF16 = mybir.dt.bfloat16

SELU_S = 1.0507009873554805
SELU_A = 1.6732632423543772

_CACHE = {}
N_CORES = 8
NPC = 128          # images per core
NPAIR = NPC // 2   # 64 image pairs per core
CH_PAIRS = int(os.environ.get("BASSK_CH_PAIRS", "16"))  # pairs per im2col chunk
NCHUNK = NPAIR // CH_PAIRS

RELU_FRAC = float(os.environ.get("BASSK_RELU_FRAC", "0.75"))  # fraction of pairs with relu on ACT
SKIP = set(os.environ.get("BASSK_SKIP", "").split(","))


def _split_multiwait(nc):
    """The walrus build here can only codegen ONE sync-wait per instruction.
    Move extra waits onto prefix no-ops on the same engine (same semantics:
    the sequencer executes the waits in program order before the op)."""
    ctr = 0
    for f in nc.m.functions:
        for blk in f.blocks:
            il = blk.instructions
            i = 0
            while i < len(il):
                ins = il[i]
                si = ins.sync_info
                waits = list(si.on_wait) if (si is not None and si.on_wait) else []
                if len(waits) > 1:
                    for w in waits[:-1]:
                        ctr += 1
                        nop = bass_rust.InstNoOp(name=f"I-mw{ctr}", ins=[], outs=[])
                        nop.engine = ins.engine
                        nop.sync_info = bass_rust.SyncInfo(on_wait=[w], on_update=[])
                        il.insert(i, nop)
                        i += 1
                    ins.sync_info = bass_rust.SyncInfo(
                        on_wait=[waits[-1]], on_update=list(si.on_update or [])
                    )
                i += 1
    return ctr


def build_nc():
    nc = bass.Bass("TRN2")
    x = nc.dram_tensor("x", (NPC, 3, 32, 32), F32, kind="ExternalInput")
    # host-packed block-diagonal conv weights (incl. bias ones-row target)
    lhsT_d = nc.dram_tensor("lhsT_host", (3, 19, 128), BF16, kind="ExternalInput")
    w2_d = nc.dram_tensor("w2_host", (4, 64), BF16, kind="ExternalInput")
    out = nc.dram_tensor("out", (NPC, 64), F32, kind="ExternalOutput")

    ones_dram = nc.inline_tensor(
        np.ones((1, 32 * CH_PAIRS * 34), dtype=ml_dtypes.bfloat16), name="ones_row"
    )
    ones_bf = nc.inline_tensor(
        np.ones((1, NPC), dtype=ml_dtypes.bfloat16), name="ones_bf"
    )
    sel_np = np.zeros((102, 4), dtype=np.float32)
    for c in range(3):
        sel_np[34 * c:34 * c + 34, c] = 1.0 / 1024.0
    sel_dram = nc.inline_tensor(sel_np, name="sel_const")

    with ExitStack() as es:
        # persistent SBUF tensors (live across both TileContexts)
        x_pad = es.enter_context(nc.sbuf_tensor("x_pad", [102, NPC, 34], BF16))
        lhsT = es.enter_context(nc.sbuf_tensor("lhsT", [19, 3 * 128], BF16))
        w2 = es.enter_context(nc.sbuf_tensor("w2", [4, 64], BF16))
        sel = es.enter_context(nc.sbuf_tensor("sel", [102, 4], F32))
        rowsums = es.enter_context(nc.sbuf_tensor("rowsums", [102, NPC], F32))
        csT = es.enter_context(nc.sbuf_tensor("csT", [4, NPC], BF16))
        gexp = es.enter_context(nc.sbuf_tensor("gexp", [128, NPAIR], F32))
        gd = es.enter_context(nc.sbuf_tensor("gd", [128, NPAIR], F32))
        gate = es.enter_context(nc.sbuf_tensor("gate", [128, NPAIR], F32))
        q_all = es.enter_context(nc.sbuf_tensor("q_all", [128, NPAIR * 512], BF16))
        res = es.enter_context(nc.sbuf_tensor("res", [128, NPAIR], F32))

        # ---- phase 0: setup (pad buffer, input load, weights, constants)
        with TileContext(nc) as tc0:
            nc.vector.memset(x_pad[:, :, :], 0.0)
            nc.sync.dma_start(out=lhsT[:, :].rearrange("k (kj o) -> k kj o", kj=3), in_=lhsT_d[:, :, :].rearrange("kj k o -> k kj o"))
            nc.sync.dma_start(out=w2[:, :], in_=w2_d[:, :])
            nc.sync.dma_start(out=sel[:, :], in_=sel_dram[:, :])
            nc.sync.dma_start(out=csT[3:4, :], in_=ones_bf[:, :])
            # x: (n,c,i,j) f32 -> x_pad[c*34 + i+1, n, j+1] bf16 (cast DMA)
            for c in range(3):
                nc.gpsimd.dma_start(
                    out=x_pad[c * 34 + 1:c * 34 + 33, :, 1:33],
                    in_=x[:, c, :, :].rearrange("n i j -> i n j"),
                )

        with TileContext(nc) as tc:
            with tc.tile_pool(name="patchp", bufs=int(os.environ.get("BASSK_PATCHBUFS", "2"))) as patch_pool, \
                 tc.tile_pool(name="workp", bufs=int(os.environ.get("BASSK_WORKBUFS", "3"))) as work_pool, \
                 tc.tile_pool(name="psump", bufs=int(os.environ.get("BASSK_PSUMBUFS", "3")), space="PSUM") as psum_pool, \
                 tc.tile_pool(name="gpsum", bufs=1, space="PSUM") as gpsum_pool:

                # ---- gate path: channel means -> sigmoid(cs @ scale_proj + sb)
                nc.vector.tensor_reduce(
                    rowsums[:, :], x_pad[:, :, :], axis=AX.X, op=ALU.add
                )
                cs_ps = gpsum_pool.tile([3, NPC], F32, tag="cs")
                nc.tensor.matmul(cs_ps[:, :], sel[:, 0:3], rowsums[:, :], start=True, stop=True)
                nc.vector.tensor_copy(csT[0:3, :], cs_ps[:, :])

                gp_ps = gpsum_pool.tile([128, NPAIR], F32, tag="gp")
                csv = csT[:, :].rearrange("k (ch half p) -> k ch half p", ch=NCHUNK, half=2)
                nc.tensor.matmul(gp_ps[0:64, :], w2[:, :], csv[:, :, 0:1, :], start=True, stop=True)
                nc.tensor.matmul(gp_ps[64:128, :], w2[:, :], csv[:, :, 1:2, :], start=True, stop=True)
                nc.scalar.activation(gexp[:, :], gp_ps[:, :], AF.Exp, scale=-1.0)
                nc.vector.tensor_scalar(gd[:, :], gexp[:, :], 1.0, None, ALU.add)
                nc.vector.reciprocal(gate[:, :], gd[:, :])

                # ---- main loop: conv + selu^2 + pair-sum ----
                for ch in range(NCHUNK):
                    # ki-unfolded patch: partition k = 9*r + 3*ki + c holds
                    # x[c, i+ki-1, :] for the r-th 8-image block of the chunk;
                    # free layout = i*(8*34) + p*34 + jpad (rows stay padded so
                    # the kj shifts become matmul rhs free offsets).
                    patch = patch_pool.tile([19, 32 * CH_PAIRS * 34], BF16, tag="patch")
                    nc.sync.dma_start(out=patch[18:19, :], in_=ones_dram[:, :])
                    for ki in range(3) if "im2col" not in SKIP else []:
                        for r in range(2):
                            for c in range(3):
                                kb = 9 * r + 3 * ki + c
                                src = x_pad[
                                    c * 34 + ki:c * 34 + ki + 32,
                                    2 * CH_PAIRS * ch + CH_PAIRS * r:
                                    2 * CH_PAIRS * ch + CH_PAIRS * (r + 1),
                                    :,
                                ]
                                dst = patch[kb:kb + 1, :].rearrange(
                                    "o (i pj) -> o i pj", pj=CH_PAIRS * 34,
                                )
                                nc.sync.dma_start(out=dst, in_=src)

                    for p in range(CH_PAIRS):
                        g = ch * CH_PAIRS + p
                        psum_t = psum_pool.tile([128, 1024], F32, tag="conv")
                        pv = patch[:, :].rearrange(
                            "k (i p j) -> k i p j", i=32, p=CH_PAIRS
                        )[:, :, p:p + 1, :]
                        if "mm" not in SKIP:
                            for kj in range(3):
                                wkj = lhsT[:, 128 * kj:128 * kj + 128]
                                rhs_e = pv[:, :, :, kj + 0:kj + 31:2]
                                rhs_o = pv[:, :, :, kj + 1:kj + 32:2]
                                nc.tensor.matmul(psum_t[:, 0:512], wkj, rhs_e,
                                                 start=(kj == 0), stop=(kj == 2))
                                nc.tensor.matmul(psum_t[:, 512:1024], wkj, rhs_o,
                                                 start=(kj == 0), stop=(kj == 2))

                        # conv out here is a~ = a/alpha (weights pre-scaled on
                        # host); e = exp(alpha * a~) = exp(a);
                        # m = min(e-1, 0); t = relu(a)/alpha + m = selu(a)/(s*alpha)
                        if "elem" in SKIP:
                            continue
                        e_sb = work_pool.tile([128, 1024], BF16, tag="e")
                        nc.scalar.activation(e_sb[:, :], psum_t[:, :], AF.Exp, scale=SELU_A)
                        m_sb = work_pool.tile([128, 1024], BF16, tag="m")
                        nc.vector.tensor_scalar(
                            m_sb[:, :], e_sb[:, :], -1.0, 0.0, ALU.add, ALU.min
                        )
                        if (p % 4) < int(RELU_FRAC * 4 + 0.5):
                            r_sb = work_pool.tile([128, 1024], BF16, tag="r")
                            nc.scalar.activation(r_sb[:, :], psum_t[:, :], AF.Relu)
                            t_sb = work_pool.tile([128, 1024], BF16, tag="t")
                            nc.vector.tensor_tensor(
                                t_sb[:, :], r_sb[:, :], m_sb[:, :], ALU.add
                            )
                        else:
                            t_sb = work_pool.tile([128, 1024], BF16, tag="t")
                            nc.vector.scalar_tensor_tensor(
                                t_sb[:, :], psum_t[:, :], 0.0, m_sb[:, :], ALU.max, ALU.add
                            )
                        u_sb = work_pool.tile([128, 1024], BF16, tag="u")
                        nc.vector.tensor_tensor(
                            u_sb[:, :], t_sb[:, :], t_sb[:, :], ALU.mult
                        )
                        nc.vector.tensor_tensor(
                            q_all[:, g * 512:(g + 1) * 512],
                            u_sb[:, 0:512], u_sb[:, 512:1024], ALU.add,
                        )

        # ---- phase B: sqrt + fused per-pair reduction, gate multiply, store
        with TileContext(nc) as tc2:
            with tc2.tile_pool(name="ph2", bufs=2) as pool2:
                # sqrt at FD=2048 (4 pairs/op) on ACT; per-pair mean via
                # cheap 4x DVE tensor_scalar accum_out reductions.
                for gg in range(0, NPAIR, 4) if "sqrt" not in SKIP else []:
                    scr = pool2.tile([128, 2048], BF16, tag="scr")
                    nc.scalar.activation(
                        scr[:, :], q_all[:, gg * 512:(gg + 4) * 512], AF.Sqrt,
                        scale=(SELU_S * SELU_A / 512.0) ** 2,
                    )
                    for k in range(4):
                        scr2 = pool2.tile([128, 512], BF16, tag="scr2")
                        nc.vector.tensor_scalar(
                            scr2[:, :], scr[:, k * 512:(k + 1) * 512],
                            1.0, 0.0, ALU.mult, ALU.add,
                            accum_out=res[:, gg + k:gg + k + 1],
                        )
                outv = pool2.tile([128, NPAIR], F32, tag="outv")
                nc.vector.tensor_tensor(outv[:, :], res[:, :], gate[:, :], ALU.mult)
                # identity store; host reorders rows (see kernel())
                nc.sync.dma_start(out=out[:, :], in_=outv[:, :])

    _split_multiwait(nc)
    return nc


def _pack_weights(weight, bias, scale_proj, scale_bias):
    # lhsT rows ordered (ki,kj,c); block-diagonal over the two images of a
    # pair; row 54 = ones-row weights = conv bias (shared by both blocks).
    # lhsT[kj][3*ki + c, o(+64)] = weight[(c,ki,kj), o]; bias rides kj=1
    w4 = weight.reshape(3, 3, 3, 64)  # (c, ki, kj, o)
    lhsT = np.zeros((3, 19, 128), dtype=np.float32)
    for kj in range(3):
        blk = w4[:, :, kj, :].transpose(1, 0, 2).reshape(9, 64)  # k' = 3*ki + c
        lhsT[kj, 0:9, 0:64] = blk
        lhsT[kj, 9:18, 64:128] = blk
    lhsT[1, 18, 0:64] = bias
    lhsT[1, 18, 64:128] = bias
    lhsT *= 1.0 / SELU_A  # conv emits a/alpha; exp undoes via scale=alpha
    w2 = np.zeros((4, 64), dtype=np.float32)
    w2[0:3] = scale_proj
    w2[3] = scale_bias
    return (lhsT.astype(ml_dtypes.bfloat16), w2.astype(ml_dtypes.bfloat16))


def kernel(x, weight, bias, scale_proj, scale_bias):
    x = np.ascontiguousarray(np.asarray(x, dtype=np.float32))
    weight = np.asarray(weight, dtype=np.float32)
    bias = np.asarray(bias, dtype=np.float32)
    scale_proj = np.asarray(scale_proj, dtype=np.float32)
    scale_bias = np.asarray(scale_bias, dtype=np.float32)

    lhsT_host, w2_host = _pack_weights(weight, bias, scale_proj, scale_bias)

    if "nc" not in _CACHE:
        _CACHE["nc"] = build_nc()
    nc = _CACHE["nc"]

    in_maps = [
        {
            "x": x[i * NPC:(i + 1) * NPC],
            "lhsT_host": lhsT_host,
            "w2_host": w2_host,
        }
        for i in range(N_CORES)
    ]
    r = run_bass_kernel_spmd(nc, in_maps, core_ids=list(range(N_CORES)))
    _CACHE["last_result"] = r
    outs = []
    for m in r.results:
        o = m["out"]  # [128 rows = (r, chan), 64 cols = (ch, p)]
        o4 = o.reshape(2, 64, NCHUNK, CH_PAIRS)  # (r, chan, ch, p)
        o4 = o4.transpose(2, 0, 3, 1)            # (ch, r, p, chan)
        outs.append(np.ascontiguousarray(o4.reshape(128, 64)))
    return np.concatenate(outs, axis=0)



# revision 5
# speedup vs baseline: 1.0004x; 1.0004x over previous
"""Trainium2 Bass kernel for nn_Model_11888469475921 (dense_cnn).

Computation (per image, 1024 images total):
  proj = conv3x3(x, weight) + bias          # (64, 32, 32), padding 1
  act  = selu(proj)
  pooled = sqrt(act[...,0::2]^2 + act[...,1::2]^2)   # LPPool1d(p=2,k=2) along W
  gate = sigmoid(mean_{H,W}(x) @ scale_proj + scale_bias)
  out  = mean_{H,Wp}(pooled) * gate         # (64,)

Strategy: pure data parallel over 8 NeuronCores (128 images each).

Conv (V2): FULL-K im2col, K=57 = 3 kj-blocks x (2 images x 9 (ki,c) rows) +
ones row.  Per chunk a base 19-row patch [19, 32*CH_PAIRS*34] bf16 is built by
transpose DMAs (as before); the kj=1/kj=2 blocks are byte-shifted copies of
the base block (patch[19k+r, jj] = base[r, jj+k]) made with ONE big contiguous
SBUF->SBUF DMA each.  One matmul per 512-pixel parity group (vs 3 accumulating
ones): PE time drops 3x.  Conv bias rides the kj=1 ones row only.

Elementwise (V2): PSUM tiles hold TWO pairs [128, 2048] to halve per-op init
overhead.  Per tile:
  e = Exp(alpha * a~)            (ACT, PSUM->SBUF bf16)
  m = min(e-1, 0)                (DVE ts, 4x)
  t = relu(a~) + m  = selu/(s*alpha), via one of three engine paths chosen
      round-robin to balance load: ACT Relu + DVE tt-add / DVE stt (1x) /
      Pool(GPSIMD) stt,
  u = t*t                        (DVE tt, 2x)
  q = u_even + u_odd             (DVE tt or Pool tt, per-tile split)
  res[:, g] = ts-accum of ACT Sqrt((s*alpha/512)^2 * q)  (fused pool+mean)
The m pass also has a Pool share.  All fractions env-tunable.

Gate: channel sums via row-reduce + selector matmul into a borrowed region of
the conv PSUM tile (no extra PSUM banks), sigmoid as 1/(1+exp(-x)).

Phases: setup / main / sqrt in separate TileContexts. _split_multiwait() keeps
every instruction at <=1 sync wait (walrus single-wait codegen limit).

Output rows are stored in (r, chan) x (chunk, p) order and permuted on host.
"""

import os
import numpy as np
import ml_dtypes
from contextlib import ExitStack

import bass_rust
import concourse.bass as bass
import concourse.mybir as mybir
from concourse.tile import TileContext
from concourse.bass_utils import run_bass_kernel_spmd

AF = mybir.ActivationFunctionType
ALU = mybir.AluOpType
AX = mybir.AxisListType
F32 = mybir.dt.float32
BF16 = mybir.dt.bfloat16

SELU_S = 1.0507009873554805
SELU_A = 1.6732632423543772

_CACHE = {}
N_CORES = 8
NPC = 128          # images per core
NPAIR = NPC // 2   # 64 image pairs per core
CH_PAIRS = 16      # pairs per im2col chunk
NCHUNK = NPAIR // CH_PAIRS
TPC = CH_PAIRS // 2        # 2-pair tiles per chunk
NTILE = NPAIR // 2         # 2-pair tiles total (32)

# engine-balance fractions (GPSIMD cannot touch PSUM, so the relu/t path is
# split only between ACT (relu + DVE tt-add) and DVE (1x stt); Pool absorbs
# the SBUF-only m and q passes)
FRAC_TA = float(os.environ.get("BASSK_TA", "0.375"))  # t via ACT relu
FRAC_MP = float(os.environ.get("BASSK_MP", "1.0"))    # m on Pool
FRAC_QP = float(os.environ.get("BASSK_QP", "0.8"))    # q on Pool


def _split_multiwait(nc):
    """The walrus build here can only codegen ONE sync-wait per instruction.
    Move extra waits onto prefix no-ops on the same engine (same semantics:
    the sequencer executes the waits in program order before the op)."""
    ctr = 0
    for f in nc.m.functions:
        for blk in f.blocks:
            il = blk.instructions
            i = 0
            while i < len(il):
                ins = il[i]
                si = ins.sync_info
                waits = list(si.on_wait) if (si is not None and si.on_wait) else []
                if len(waits) > 1:
                    for w in waits[:-1]:
                        ctr += 1
                        nop = bass_rust.InstNoOp(name=f"I-mw{ctr}", ins=[], outs=[])
                        nop.engine = ins.engine
                        nop.sync_info = bass_rust.SyncInfo(on_wait=[w], on_update=[])
                        il.insert(i, nop)
                        i += 1
                    ins.sync_info = bass_rust.SyncInfo(
                        on_wait=[waits[-1]], on_update=list(si.on_update or [])
                    )
                i += 1
    return ctr


def _frac_select(idx, total, frac):
    """Bresenham-style even spread: True for ~frac of indices."""
    return int((idx + 1) * frac) - int(idx * frac) > 0


def build_nc():
    nc = bass.Bass("TRN2")
    x = nc.dram_tensor("x", (NPC, 3, 32, 32), F32, kind="ExternalInput")
    # host-packed block-diagonal conv weights (incl. bias ones-row target)
    lhsT_d = nc.dram_tensor("lhsT_host", (57, 128), BF16, kind="ExternalInput")
    w2_d = nc.dram_tensor("w2_host", (4, 64), BF16, kind="ExternalInput")
    out = nc.dram_tensor("out", (NPC, 64), F32, kind="ExternalOutput")

    ones_dram = nc.inline_tensor(
        np.ones((1, 32 * CH_PAIRS * 34), dtype=ml_dtypes.bfloat16), name="ones_row"
    )
    ones_bf = nc.inline_tensor(
        np.ones((1, NPC), dtype=ml_dtypes.bfloat16), name="ones_bf"
    )
    sel_np = np.zeros((102, 4), dtype=np.float32)
    for c in range(3):
        sel_np[34 * c:34 * c + 34, c] = 1.0 / 1024.0
    sel_dram = nc.inline_tensor(sel_np, name="sel_const")

    FREE = 32 * CH_PAIRS * 34  # flat free size of one patch row block

    with ExitStack() as es:
        # persistent SBUF tensors (live across TileContexts)
        x_pad = es.enter_context(nc.sbuf_tensor("x_pad", [102, NPC, 34], BF16))
        lhsT = es.enter_context(nc.sbuf_tensor("lhsT", [57, 128], BF16))
        w2 = es.enter_context(nc.sbuf_tensor("w2", [4, 64], BF16))
        sel = es.enter_context(nc.sbuf_tensor("sel", [102, 4], F32))
        rowsums = es.enter_context(nc.sbuf_tensor("rowsums", [102, NPC], F32))
        csT = es.enter_context(nc.sbuf_tensor("csT", [4, NPC], BF16))
        gexp = es.enter_context(nc.sbuf_tensor("gexp", [128, NPAIR], F32))
        gd = es.enter_context(nc.sbuf_tensor("gd", [128, NPAIR], F32))
        gate = es.enter_context(nc.sbuf_tensor("gate", [128, NPAIR], F32))
        q_all = es.enter_context(nc.sbuf_tensor("q_all", [128, NPAIR * 512], BF16))
        res = es.enter_context(nc.sbuf_tensor("res", [128, NPAIR], F32))

        # ---- phase 0: setup (pad buffer, input load, weights, constants)
        with TileContext(nc) as tc0:
            nc.vector.memset(x_pad[:, :, :], 0.0)
            nc.sync.dma_start(out=lhsT[:, :], in_=lhsT_d[:, :])
            nc.sync.dma_start(out=w2[:, :], in_=w2_d[:, :])
            nc.sync.dma_start(out=sel[:, :], in_=sel_dram[:, :])
            nc.sync.dma_start(out=csT[3:4, :], in_=ones_bf[:, :])
            # x: (n,c,i,j) f32 -> x_pad[c*34 + i+1, n, j+1] bf16 (cast DMA)
            for c in range(3):
                nc.gpsimd.dma_start(
                    out=x_pad[c * 34 + 1:c * 34 + 33, :, 1:33],
                    in_=x[:, c, :, :].rearrange("n i j -> i n j"),
                )

        with TileContext(nc) as tc:
            with tc.tile_pool(name="patchp", bufs=int(os.environ.get("BASSK_PATCHBUFS", "2"))) as patch_pool, \
                 tc.tile_pool(name="workp", bufs=int(os.environ.get("BASSK_WORKBUFS", "3"))) as work_pool, \
                 tc.tile_pool(name="psump", bufs=2, space="PSUM") as psum_pool:

                # ---- gate path: channel means -> sigmoid(cs @ scale_proj + sb)
                # (borrows regions of the first conv-psum-pool tile: no extra
                # PSUM banks needed)
                nc.vector.tensor_reduce(
                    rowsums[:, :], x_pad[:, :, :], axis=AX.X, op=ALU.add
                )
                gps = psum_pool.tile([128, 2048], F32, tag="conv")
                cs_ps = gps[0:3, 0:NPC]
                nc.tensor.matmul(cs_ps, sel[:, 0:3], rowsums[:, :], start=True, stop=True)
                nc.vector.tensor_copy(csT[0:3, :], cs_ps)

                gp_ps = gps[:, 256:256 + NPAIR]
                csv = csT[:, :].rearrange("k (ch half p) -> k ch half p", ch=NCHUNK, half=2)
                nc.tensor.matmul(gp_ps[0:64, :], w2[:, :], csv[:, :, 0:1, :], start=True, stop=True)
                nc.tensor.matmul(gp_ps[64:128, :], w2[:, :], csv[:, :, 1:2, :], start=True, stop=True)
                nc.scalar.activation(gexp[:, :], gp_ps, AF.Exp, scale=-1.0)
                nc.vector.tensor_scalar(gd[:, :], gexp[:, :], 1.0, None, ALU.add)
                nc.vector.reciprocal(gate[:, :], gd[:, :])

                # ---- main loop: conv + selu^2 + pair-sum ----
                for ch in range(NCHUNK):
                    # Base ki-unfolded patch block (19 rows): partition
                    # k = 9*r + 3*ki + c holds x[c, i+ki-1, row] for the r-th
                    # 8-image half of the chunk; free layout = i*(P*34) +
                    # p*34 + jpad.  Rows 19..37 / 38..56 are the same block
                    # shifted left by 1 / 2 free elements (kj=1 / kj=2), so
                    # ONE K=57 matmul covers all of (c, ki, kj).
                    patch = patch_pool.tile([57, FREE], BF16, tag="patch")
                    nc.sync.dma_start(out=patch[18:19, :], in_=ones_dram[:, :])
                    for ki in range(3):
                        for r in range(2):
                            for c in range(3):
                                kb = 9 * r + 3 * ki + c
                                src = x_pad[
                                    c * 34 + ki:c * 34 + ki + 32,
                                    2 * CH_PAIRS * ch + CH_PAIRS * r:
                                    2 * CH_PAIRS * ch + CH_PAIRS * (r + 1),
                                    :,
                                ]
                                dst = patch[kb:kb + 1, :].rearrange(
                                    "o (i pj) -> o i pj", pj=CH_PAIRS * 34,
                                )
                                nc.sync.dma_start(out=dst, in_=src)
                    # kj=1 / kj=2 blocks: shifted copies of the base block
                    nc.sync.dma_start(out=patch[19:38, 0:FREE - 1],
                                      in_=patch[0:19, 1:FREE])
                    nc.sync.dma_start(out=patch[38:57, 0:FREE - 2],
                                      in_=patch[0:19, 2:FREE])

                    pv = patch[:, :].rearrange(
                        "k (i p j) -> k i p j", i=32, p=CH_PAIRS
                    )
                    for tp in range(TPC):
                        ti = ch * TPC + tp  # global 2-pair tile index
                        g0 = ch * CH_PAIRS + 2 * tp  # first pair in tile
                        psum_t = psum_pool.tile([128, 2048], F32, tag="conv")
                        for half in range(2):
                            p = 2 * tp + half
                            rhs_e = pv[:, :, p:p + 1, 0:31:2]
                            rhs_o = pv[:, :, p:p + 1, 1:32:2]
                            nc.tensor.matmul(
                                psum_t[:, 1024 * half:1024 * half + 512],
                                lhsT[:, :], rhs_e, start=True, stop=True)
                            nc.tensor.matmul(
                                psum_t[:, 1024 * half + 512:1024 * half + 1024],
                                lhsT[:, :], rhs_o, start=True, stop=True)

                        # psum holds a~ = a/alpha (weights pre-scaled on host);
                        # e = exp(alpha*a~) = exp(a); m = min(e-1, 0);
                        # t = relu(a~) + m = selu(a)/(s*alpha)
                        e_sb = work_pool.tile([128, 2048], BF16, tag="e")
                        nc.scalar.activation(e_sb[:, :], psum_t[:, :], AF.Exp, scale=SELU_A)
                        m_sb = work_pool.tile([128, 2048], BF16, tag="m")
                        meng = nc.gpsimd if _frac_select(ti, NTILE, FRAC_MP) else nc.vector
                        meng.tensor_scalar(
                            m_sb[:, :], e_sb[:, :], -1.0, 0.0, ALU.add, ALU.min
                        )
                        t_sb = work_pool.tile([128, 2048], BF16, tag="t")
                        if _frac_select(ti, NTILE, FRAC_TA):
                            r_sb = work_pool.tile([128, 2048], BF16, tag="r")
                            nc.scalar.activation(r_sb[:, :], psum_t[:, :], AF.Relu)
                            nc.vector.tensor_tensor(
                                t_sb[:, :], r_sb[:, :], m_sb[:, :], ALU.add
                            )
                        else:
                            nc.vector.scalar_tensor_tensor(
                                t_sb[:, :], psum_t[:, :], 0.0, m_sb[:, :], ALU.max, ALU.add
                            )
                        u_sb = work_pool.tile([128, 2048], BF16, tag="u")
                        nc.vector.tensor_tensor(
                            u_sb[:, :], t_sb[:, :], t_sb[:, :], ALU.mult
                        )
                        # u = (pairA: e 512 | o 512, pairB: e 512 | o 512)
                        u4 = u_sb[:, :].rearrange("z (pr par x) -> z pr par x", pr=2, par=2)
                        qeng = nc.gpsimd if _frac_select(ti, NTILE, FRAC_QP) else nc.vector
                        qeng.tensor_tensor(
                            q_all[:, g0 * 512:(g0 + 2) * 512],
                            u4[:, :, 0, :], u4[:, :, 1, :], ALU.add,
                        )

        # ---- phase B: sqrt + fused per-pair reduction, gate multiply, store
        with TileContext(nc) as tc2:
            with tc2.tile_pool(name="ph2", bufs=2) as pool2:
                # sqrt at FD=4096 (8 pairs/op) on ACT; per-pair mean via
                # cheap 4x DVE tensor_scalar accum_out reductions.
                for gg in range(0, NPAIR, 8):
                    scr = pool2.tile([128, 4096], BF16, tag="scr")
                    nc.scalar.activation(
                        scr[:, :], q_all[:, gg * 512:(gg + 8) * 512], AF.Sqrt,
                        scale=(SELU_S * SELU_A / 512.0) ** 2,
                    )
                    for k in range(8):
                        scr2 = pool2.tile([128, 512], BF16, tag="scr2")
                        nc.vector.tensor_scalar(
                            scr2[:, :], scr[:, k * 512:(k + 1) * 512],
                            1.0, 0.0, ALU.mult, ALU.add,
                            accum_out=res[:, gg + k:gg + k + 1],
                        )
                outv = pool2.tile([128, NPAIR], F32, tag="outv")
                nc.vector.tensor_tensor(outv[:, :], res[:, :], gate[:, :], ALU.mult)
                # identity store; host reorders rows (see kernel())
                nc.sync.dma_start(out=out[:, :], in_=outv[:, :])

    _split_multiwait(nc)
    return nc


def _pack_weights(weight, bias, scale_proj, scale_bias):
    # lhsT rows: 19*kj + 9*r + 3*ki + c ; block-diagonal over the two images
    # of a pair (cols 0:64 / 64:128); row 37 = kj=1 ones-row = conv bias.
    w4 = weight.reshape(3, 3, 3, 64)  # (c, ki, kj, o)
    lhsT = np.zeros((57, 128), dtype=np.float32)
    for kj in range(3):
        blk = w4[:, :, kj, :].transpose(1, 0, 2).reshape(9, 64)  # k' = 3*ki + c
        lhsT[19 * kj + 0:19 * kj + 9, 0:64] = blk
        lhsT[19 * kj + 9:19 * kj + 18, 64:128] = blk
    lhsT[37, 0:64] = bias
    lhsT[37, 64:128] = bias
    lhsT *= 1.0 / SELU_A  # conv emits a/alpha; exp undoes via scale=alpha
    w2 = np.zeros((4, 64), dtype=np.float32)
    w2[0:3] = scale_proj
    w2[3] = scale_bias
    return (lhsT.astype(ml_dtypes.bfloat16), w2.astype(ml_dtypes.bfloat16))


def kernel(x, weight, bias, scale_proj, scale_bias):
    x = np.ascontiguousarray(np.asarray(x, dtype=np.float32))
    weight = np.asarray(weight, dtype=np.float32)
    bias = np.asarray(bias, dtype=np.float32)
    scale_proj = np.asarray(scale_proj, dtype=np.float32)
    scale_bias = np.asarray(scale_bias, dtype=np.float32)

    lhsT_host, w2_host = _pack_weights(weight, bias, scale_proj, scale_bias)

    if "nc" not in _CACHE:
        _CACHE["nc"] = build_nc()
    nc = _CACHE["nc"]

    in_maps = [
        {
            "x": x[i * NPC:(i + 1) * NPC],
            "lhsT_host": lhsT_host,
            "w2_host": w2_host,
        }
        for i in range(N_CORES)
    ]
    r = run_bass_kernel_spmd(nc, in_maps, core_ids=list(range(N_CORES)))
    _CACHE["last_result"] = r
    outs = []
    for m in r.results:
        o = m["out"]  # [128 rows = (r, chan), 64 cols = (ch, p)]
        o4 = o.reshape(2, 64, NCHUNK, CH_PAIRS)  # (r, chan, ch, p)
        o4 = o4.transpose(2, 0, 3, 1)            # (ch, r, p, chan)
        outs.append(np.ascontiguousarray(o4.reshape(128, 64)))
    return np.concatenate(outs, axis=0)


# revision 9
# speedup vs baseline: 1.1493x; 1.1489x over previous
"""Trainium2 Bass kernel for nn_Model_11888469475921 (dense_cnn).

Computation (per image, 1024 images total):
  proj = conv3x3(x, weight) + bias          # (64, 32, 32), padding 1
  act  = selu(proj)
  pooled = sqrt(act[...,0::2]^2 + act[...,1::2]^2)   # LPPool1d(p=2,k=2) along W
  gate = sigmoid(mean_{H,W}(x) @ scale_proj + scale_bias)
  out  = mean_{H,Wp}(pooled) * gate         # (64,)

Strategy: pure data parallel over 8 NeuronCores (128 images each).

Conv: FULL-K im2col, K=57 = 3 kj-blocks x (2 images x 9 (ki,c) rows) + ones
row.  Per chunk a base 19-row patch [19, 32*CH_PAIRS*34] bf16 is built by
transpose DMAs; the kj=1/kj=2 blocks are shifted copies of the base block
(patch[19k+r, jj] = base[r, jj+k]) made with ONE big contiguous SBUF->SBUF DMA
each.  One K=57 matmul per 512-pixel parity group: PE time is 3x less than
the 3-accumulating-matmuls scheme.  Conv bias rides the kj=1 ones row only.

Elementwise: PSUM tiles hold TWO pairs [128, 2048] to amortize per-op init.
Per tile:
  e = Exp(alpha * a~)            (ACT, PSUM->SBUF bf16)
  m = min(e-1, 0)                (DVE ts, 4x mode)
  t = relu(a~) + m  = selu/(s*alpha): ACT Relu + DVE tt-add for FRAC_TA of
      tiles, DVE stt (1x, reads PSUM) for the rest  (GPSIMD cannot touch PSUM)
  u = t*t                        (DVE tt, 2x)
  q = u_even + u_odd             (Pool/DVE tt per FRAC_QP; off critical path)
  sqrt+mean: per chunk, ACT Sqrt at FD=4096 + per-pair DVE ts-accum --
      runs inside the main loop so it pipelines behind the next chunk.
Gate: computed in the tail phase (channel sums via row-reduce early + selector
matmul, sigmoid as 1/(1+exp(-x))), overlapping the last chunk's drain.

_split_multiwait() keeps every instruction at <=1 sync wait (walrus
single-wait codegen limit).

Output rows are stored in (r, chan) x (chunk, p) order and permuted on host.
"""

import os
import numpy as np
import ml_dtypes
from contextlib import ExitStack

import bass_rust
import concourse.bass as bass
import concourse.mybir as mybir
from concourse.tile import TileContext
from concourse.bass_utils import run_bass_kernel_spmd

AF = mybir.ActivationFunctionType
ALU = mybir.AluOpType
AX = mybir.AxisListType
F32 = mybir.dt.float32
BF16 = mybir.dt.bfloat16

SELU_S = 1.0507009873554805
SELU_A = 1.6732632423543772

_CACHE = {}
N_CORES = 8
NPC = 128          # images per core
NPAIR = NPC // 2   # 64 image pairs per core
CH_PAIRS = 16      # pairs per im2col chunk
NCHUNK = NPAIR // CH_PAIRS
TPC = CH_PAIRS // 2        # 2-pair tiles per chunk
NTILE = NPAIR // 2         # 2-pair tiles total (32)

FRAC_TA = float(os.environ.get("BASSK_TA", "0.52"))   # t via ACT relu
FRAC_MP = float(os.environ.get("BASSK_MP", "0.0"))    # m on Pool
FRAC_QP = float(os.environ.get("BASSK_QP", "1.0"))    # q on Pool


def _split_multiwait(nc):
    """The walrus build here can only codegen ONE sync-wait per instruction.
    Move extra waits onto prefix no-ops on the same engine (same semantics:
    the sequencer executes the waits in program order before the op)."""
    ctr = 0
    for f in nc.m.functions:
        for blk in f.blocks:
            il = blk.instructions
            i = 0
            while i < len(il):
                ins = il[i]
                si = ins.sync_info
                waits = list(si.on_wait) if (si is not None and si.on_wait) else []
                if len(waits) > 1:
                    for w in waits[:-1]:
                        ctr += 1
                        nop = bass_rust.InstNoOp(name=f"I-mw{ctr}", ins=[], outs=[])
                        nop.engine = ins.engine
                        nop.sync_info = bass_rust.SyncInfo(on_wait=[w], on_update=[])
                        il.insert(i, nop)
                        i += 1
                    ins.sync_info = bass_rust.SyncInfo(
                        on_wait=[waits[-1]], on_update=list(si.on_update or [])
                    )
                i += 1
    return ctr


def _frac_select(idx, total, frac):
    """Bresenham-style even spread: True for ~frac of indices."""
    return int((idx + 1) * frac) - int(idx * frac) > 0


def build_nc():
    nc = bass.Bass("TRN2")
    x = nc.dram_tensor("x", (NPC, 3, 32, 32), F32, kind="ExternalInput")
    # host-packed block-diagonal conv weights (incl. bias ones-row target)
    lhsT_d = nc.dram_tensor("lhsT_host", (57, 128), BF16, kind="ExternalInput")
    w2_d = nc.dram_tensor("w2_host", (4, 64), BF16, kind="ExternalInput")
    out = nc.dram_tensor("out", (NPC, 64), F32, kind="ExternalOutput")

    ones_dram = nc.inline_tensor(
        np.ones((1, 32 * CH_PAIRS * 34), dtype=ml_dtypes.bfloat16), name="ones_row"
    )
    ones_bf = nc.inline_tensor(
        np.ones((1, NPC), dtype=ml_dtypes.bfloat16), name="ones_bf"
    )
    zeros_dram = nc.inline_tensor(
        np.zeros((6, NPC * 34), dtype=ml_dtypes.bfloat16), name="zeros_rows"
    )
    sel_np = np.zeros((102, 4), dtype=np.float32)
    for c in range(3):
        sel_np[34 * c:34 * c + 34, c] = 1.0 / 1024.0
    sel_dram = nc.inline_tensor(sel_np, name="sel_const")

    FREE = 32 * CH_PAIRS * 34  # flat free size of one patch row block

    with ExitStack() as es:
        # persistent SBUF tensors (live across TileContexts)
        x_pad = es.enter_context(nc.sbuf_tensor("x_pad", [102, NPC, 34], BF16))
        lhsT = es.enter_context(nc.sbuf_tensor("lhsT", [57, 128], BF16))
        w2 = es.enter_context(nc.sbuf_tensor("w2", [4, 64], BF16))
        sel = es.enter_context(nc.sbuf_tensor("sel", [102, 4], F32))
        rowsums = es.enter_context(nc.sbuf_tensor("rowsums", [102, NPC], F32))
        csT = es.enter_context(nc.sbuf_tensor("csT", [4, NPC], BF16))
        gexp = es.enter_context(nc.sbuf_tensor("gexp", [128, NPAIR], F32))
        gd = es.enter_context(nc.sbuf_tensor("gd", [128, NPAIR], F32))
        gate = es.enter_context(nc.sbuf_tensor("gate", [128, NPAIR], F32))
        q_all = es.enter_context(nc.sbuf_tensor("q_all", [128, CH_PAIRS * 2 * 512], BF16))
        res = es.enter_context(nc.sbuf_tensor("res", [128, NPAIR], F32))

        # ---- phase 0: setup (pad zeroing, input load, weights, constants)
        with TileContext(nc) as tc0:
            # zero only the padding: the two j-pad columns (DVE, tiny free
            # size) and the six i-pad rows (DMA from an inline zeros tensor).
            nc.vector.memset(x_pad[:, :, 0:1], 0.0)
            nc.vector.memset(x_pad[:, :, 33:34], 0.0)
            nc.sync.dma_start(
                out=x_pad[0:102:34, :, :], in_=zeros_dram[0:3, :].rearrange(
                    "c (n j) -> c n j", j=34),
            )
            nc.sync.dma_start(
                out=x_pad[33:102:34, :, :], in_=zeros_dram[3:6, :].rearrange(
                    "c (n j) -> c n j", j=34),
            )
            nc.sync.dma_start(out=lhsT[:, :], in_=lhsT_d[:, :])
            nc.sync.dma_start(out=w2[:, :], in_=w2_d[:, :])
            nc.sync.dma_start(out=sel[:, :], in_=sel_dram[:, :])
            nc.sync.dma_start(out=csT[3:4, :], in_=ones_bf[:, :])
            # x: (n,c,i,j) f32 -> x_pad[c*34 + i+1, n, j+1] bf16 (cast DMA)
            for c in range(3):
                nc.gpsimd.dma_start(
                    out=x_pad[c * 34 + 1:c * 34 + 33, :, 1:33],
                    in_=x[:, c, :, :].rearrange("n i j -> i n j"),
                )

        with TileContext(nc) as tc:
            with tc.tile_pool(name="patchp", bufs=int(os.environ.get("BASSK_PATCHBUFS", "2"))) as patch_pool, \
                 tc.tile_pool(name="workp", bufs=int(os.environ.get("BASSK_WORKBUFS", "3"))) as work_pool, \
                 tc.tile_pool(name="sqp", bufs=2) as sq_pool, \
                 tc.tile_pool(name="psump", bufs=2, space="PSUM") as psum_pool:

                # gate inputs: per-(c,i)-row sums (used in the tail phase)
                nc.vector.tensor_reduce(
                    rowsums[:, :], x_pad[:, :, :], axis=AX.X, op=ALU.add
                )

                # ---- main loop: conv + selu^2 + pair-sum + sqrt-mean ----
                for ch in range(NCHUNK):
                    # Base ki-unfolded patch block (19 rows): partition
                    # k = 9*r + 3*ki + c holds x[c, i+ki-1, row] for the r-th
                    # 8-image half of the chunk; free layout = i*(P*34) +
                    # p*34 + jpad.  Rows 19..37 / 38..56 are the same block
                    # shifted left by 1 / 2 free elements (kj=1 / kj=2), so
                    # ONE K=57 matmul covers all of (c, ki, kj).
                    patch = patch_pool.tile([57, FREE], BF16, tag="patch")
                    nc.sync.dma_start(out=patch[18:19, :], in_=ones_dram[:, :])
                    for ki in range(3):
                        for r in range(2):
                            for c in range(3):
                                kb = 9 * r + 3 * ki + c
                                src = x_pad[
                                    c * 34 + ki:c * 34 + ki + 32,
                                    2 * CH_PAIRS * ch + CH_PAIRS * r:
                                    2 * CH_PAIRS * ch + CH_PAIRS * (r + 1),
                                    :,
                                ]
                                dst = patch[kb:kb + 1, :].rearrange(
                                    "o (i pj) -> o i pj", pj=CH_PAIRS * 34,
                                )
                                nc.sync.dma_start(out=dst, in_=src)
                    # kj=1 / kj=2 blocks: shifted copies of the base block
                    nc.sync.dma_start(out=patch[19:38, 0:FREE - 1],
                                      in_=patch[0:19, 1:FREE])
                    nc.sync.dma_start(out=patch[38:57, 0:FREE - 2],
                                      in_=patch[0:19, 2:FREE])

                    pv = patch[:, :].rearrange(
                        "k (i p j) -> k i p j", i=32, p=CH_PAIRS
                    )
                    for tp in range(TPC):
                        ti = ch * TPC + tp  # global 2-pair tile index
                        gq = 2 * tp        # first pair of tile, within chunk
                        psum_t = psum_pool.tile([128, 2048], F32, tag="conv")
                        for half in range(2):
                            p = 2 * tp + half
                            rhs_e = pv[:, :, p:p + 1, 0:31:2]
                            rhs_o = pv[:, :, p:p + 1, 1:32:2]
                            nc.tensor.matmul(
                                psum_t[:, 1024 * half:1024 * half + 512],
                                lhsT[:, :], rhs_e, start=True, stop=True)
                            nc.tensor.matmul(
                                psum_t[:, 1024 * half + 512:1024 * half + 1024],
                                lhsT[:, :], rhs_o, start=True, stop=True)

                        # psum holds a~ = a/alpha (weights pre-scaled on host);
                        # e = exp(alpha*a~) = exp(a); m = min(e-1, 0);
                        # t = relu(a~) + m = selu(a)/(s*alpha)
                        e_sb = work_pool.tile([128, 2048], BF16, tag="e")
                        nc.scalar.activation(e_sb[:, :], psum_t[:, :], AF.Exp, scale=SELU_A)
                        m_sb = work_pool.tile([128, 2048], BF16, tag="m")
                        meng = nc.gpsimd if _frac_select(ti, NTILE, FRAC_MP) else nc.vector
                        meng.tensor_scalar(
                            m_sb[:, :], e_sb[:, :], -1.0, 0.0, ALU.add, ALU.min
                        )
                        t_sb = work_pool.tile([128, 2048], BF16, tag="t")
                        if _frac_select(ti, NTILE, FRAC_TA):
                            r_sb = work_pool.tile([128, 2048], BF16, tag="r")
                            nc.scalar.activation(r_sb[:, :], psum_t[:, :], AF.Relu)
                            nc.vector.tensor_tensor(
                                t_sb[:, :], r_sb[:, :], m_sb[:, :], ALU.add
                            )
                        else:
                            nc.vector.scalar_tensor_tensor(
                                t_sb[:, :], psum_t[:, :], 0.0, m_sb[:, :], ALU.max, ALU.add
                            )
                        u_sb = work_pool.tile([128, 2048], BF16, tag="u")
                        nc.vector.tensor_tensor(
                            u_sb[:, :], t_sb[:, :], t_sb[:, :], ALU.mult
                        )
                        # u = (pairA: e 512 | o 512, pairB: e 512 | o 512)
                        u4 = u_sb[:, :].rearrange("z (pr par x) -> z pr par x", pr=2, par=2)
                        qeng = nc.gpsimd if _frac_select(ti, NTILE, FRAC_QP) else nc.vector
                        qb = (ch % 2) * CH_PAIRS  # double-buffer q_all by chunk
                        qeng.tensor_tensor(
                            q_all[:, (qb + gq) * 512:(qb + gq + 2) * 512],
                            u4[:, :, 0, :], u4[:, :, 1, :], ALU.add,
                        )

                    # chunk tail: sqrt at FD=4096 (8 pairs/op) on ACT, then
                    # per-pair mean via 4x DVE ts accum_out -- pipelines
                    # behind the next chunk's conv/elementwise.
                    qb = (ch % 2) * CH_PAIRS
                    for gg in range(0, CH_PAIRS, 8):
                        scr = sq_pool.tile([128, 4096], BF16, tag="scr")
                        nc.scalar.activation(
                            scr[:, :], q_all[:, (qb + gg) * 512:(qb + gg + 8) * 512],
                            AF.Sqrt,
                            scale=(SELU_S * SELU_A / 512.0) ** 2,
                        )
                        for k in range(8):
                            scr2 = sq_pool.tile([128, 512], BF16, tag="scr2")
                            nc.vector.tensor_scalar(
                                scr2[:, :], scr[:, k * 512:(k + 1) * 512],
                                1.0, 0.0, ALU.mult, ALU.add,
                                accum_out=res[:, ch * CH_PAIRS + gg + k:
                                              ch * CH_PAIRS + gg + k + 1],
                            )

        # ---- tail phase: gate (sigmoid path) + multiply + store
        with TileContext(nc) as tc2:
            with tc2.tile_pool(name="ph2", bufs=1) as pool2, \
                 tc2.tile_pool(name="gps", bufs=1, space="PSUM") as gpsum_pool:
                cs_ps = gpsum_pool.tile([3, NPC], F32, tag="cs")
                nc.tensor.matmul(cs_ps[:, :], sel[:, 0:3], rowsums[:, :], start=True, stop=True)
                nc.vector.tensor_copy(csT[0:3, :], cs_ps[:, :])

                gp_ps = gpsum_pool.tile([128, NPAIR], F32, tag="gp")
                csv = csT[:, :].rearrange("k (ch half p) -> k ch half p", ch=NCHUNK, half=2)
                nc.tensor.matmul(gp_ps[0:64, :], w2[:, :], csv[:, :, 0:1, :], start=True, stop=True)
                nc.tensor.matmul(gp_ps[64:128, :], w2[:, :], csv[:, :, 1:2, :], start=True, stop=True)
                nc.scalar.activation(gexp[:, :], gp_ps[:, :], AF.Exp, scale=-1.0)
                nc.vector.tensor_scalar(gd[:, :], gexp[:, :], 1.0, None, ALU.add)
                nc.vector.reciprocal(gate[:, :], gd[:, :])

                outv = pool2.tile([128, NPAIR], F32, tag="outv")
                nc.vector.tensor_tensor(outv[:, :], res[:, :], gate[:, :], ALU.mult)
                # identity store; host reorders rows (see kernel())
                nc.sync.dma_start(out=out[:, :], in_=outv[:, :])

    _split_multiwait(nc)
    return nc


def _pack_weights(weight, bias, scale_proj, scale_bias):
    # lhsT rows: 19*kj + 9*r + 3*ki + c ; block-diagonal over the two images
    # of a pair (cols 0:64 / 64:128); row 37 = kj=1 ones-row = conv bias.
    w4 = weight.reshape(3, 3, 3, 64)  # (c, ki, kj, o)
    lhsT = np.zeros((57, 128), dtype=np.float32)
    for kj in range(3):
        blk = w4[:, :, kj, :].transpose(1, 0, 2).reshape(9, 64)  # k' = 3*ki + c
        lhsT[19 * kj + 0:19 * kj + 9, 0:64] = blk
        lhsT[19 * kj + 9:19 * kj + 18, 64:128] = blk
    lhsT[37, 0:64] = bias
    lhsT[37, 64:128] = bias
    lhsT *= 1.0 / SELU_A  # conv emits a/alpha; exp undoes via scale=alpha
    w2 = np.zeros((4, 64), dtype=np.float32)
    w2[0:3] = scale_proj
    w2[3] = scale_bias
    return (lhsT.astype(ml_dtypes.bfloat16), w2.astype(ml_dtypes.bfloat16))


def kernel(x, weight, bias, scale_proj, scale_bias):
    x = np.ascontiguousarray(np.asarray(x, dtype=np.float32))
    weight = np.asarray(weight, dtype=np.float32)
    bias = np.asarray(bias, dtype=np.float32)
    scale_proj = np.asarray(scale_proj, dtype=np.float32)
    scale_bias = np.asarray(scale_bias, dtype=np.float32)

    lhsT_host, w2_host = _pack_weights(weight, bias, scale_proj, scale_bias)

    if "nc" not in _CACHE:
        _CACHE["nc"] = build_nc()
    nc = _CACHE["nc"]

    in_maps = [
        {
            "x": x[i * NPC:(i + 1) * NPC],
            "lhsT_host": lhsT_host,
            "w2_host": w2_host,
        }
        for i in range(N_CORES)
    ]
    r = run_bass_kernel_spmd(nc, in_maps, core_ids=list(range(N_CORES)))
    _CACHE["last_result"] = r
    outs = []
    for m in r.results:
        o = m["out"]  # [128 rows = (r, chan), 64 cols = (ch, p)]
        o4 = o.reshape(2, 64, NCHUNK, CH_PAIRS)  # (r, chan, ch, p)
        o4 = o4.transpose(2, 0, 3, 1)            # (ch, r, p, chan)
        outs.append(np.ascontiguousarray(o4.reshape(128, 64)))
    return np.concatenate(outs, axis=0)
